# revision 16
# baseline (speedup 1.0000x reference)
"""Trainium2 Bass kernel for the CustomLSTM problem.

Contract: kernel(**inputs) takes the FULL unsharded numpy inputs
(x [4096,16,512] f32, per-gate weights/biases) and returns the FULL
output h_last [4096, 1024] f32.

Strategy (data-parallel over 8 NeuronCores):
  - shard batch B=4096 -> 512 per core; replicate weights.
  - per core, per timestep t, compute fused gates in transposed layout
    gT [4H=4096, B=512] as one PSUM accumulation per 128-row gate tile.
  - mixed precision over timesteps: early steps run both projections in
    fp8e4m3 with MatmulPerfMode.DoubleRow (2x128 contraction per
    instruction at 0.5 cycles/row -> ~4x bf16 matmul throughput); the
    last few steps run in bf16 because LSTM errors injected early decay
    through the forget gates while late-step errors hit the output
    directly.  Schedule: x-part bf16 for the last KX steps, h-part bf16
    for the last KHB steps.
  - resident SBUF weights: W8/U8 (fp8) + Wb (bf16).  The bf16 U does
    not fit on top of those, so it is streamed column-by-column from
    DRAM during the bf16 tail steps (gt-major host layout so each
    column is one contiguous-per-partition DMA).
  - gates run on ScalarE straight out of PSUM with the per-gate bias
    applied via the activation instruction's bias operand; sigmoid and
    tanh share one activation table set so the c-gate uses real Tanh.
  - the element-wise chain is 4 DVE ops per 128-row slice
    (t1=f*c, t2=i*ch, c=t1+t2, h=o*tanh(c)) in bf16 (2x DVE mode);
    c stays bf16 in SBUF; the final step writes h into an f32 tile that
    is DMA'd out.
"""

import numpy as np
import ml_dtypes

import concourse.bacc as bacc
import concourse.mybir as mybir
from concourse.tile import TileContext
from concourse.bass_utils import run_bass_kernel_spmd

F32 = mybir.dt.float32
BF16 = mybir.dt.bfloat16
F8 = mybir.dt.float8e4
AF = mybir.ActivationFunctionType
DR = mybir.MatmulPerfMode.DoubleRow

B, T, D, H = 4096, 16, 512, 1024
NCORES = 8
BL = B // NCORES          # batch per core
G = 4 * H                 # fused gate dim
KD = D // 128             # x contraction tiles
KH = H // 128             # h contraction tiles
NGT = G // 128            # gate tiles

KX = 4                    # last KX steps: x-part in bf16
KHB = 1                   # last KHB steps: h-part in bf16


def build_lstm(nc, reps=1, kx=KX, khb=KHB, bufs_g=9, bufs_x=2, bufs_h=3,
               bufs_tmp=3, bufs_us=8, t_steps=None, nodep=False):
    TS = t_steps if t_steps is not None else T
    x8_d = nc.declare_dram_parameter("x8", [T * D, BL], F8, isOutput=False)
    xb_d = nc.declare_dram_parameter("xb", [T * D, BL], BF16, isOutput=False)
    w8_d = nc.declare_dram_parameter("w8", [128, KD * G], F8, isOutput=False)
    u8_d = nc.declare_dram_parameter("u8", [128, KH * G], F8, isOutput=False)
    wb_d = nc.declare_dram_parameter("wb", [128, KD * G], BF16, isOutput=False)
    ubs_d = nc.declare_dram_parameter("ubs", [128, NGT * KH * 128], BF16,
                                      isOutput=False)
    b_d = nc.declare_dram_parameter("b", [128, NGT], F32, isOutput=False)
    out_d = nc.declare_dram_parameter("h_out", [H, BL], F32, isOutput=True)

    def xprec(t):
        return '8' if t < TS - kx else 'bf'

    def hprec(t):
        # precision of the h-projection performed AT step t (t >= 1)
        return '8' if t < TS - khb else 'bf'

    with TileContext(nc) as tc:
        with tc.tile_pool(name="const", bufs=1) as cpool, \
             tc.tile_pool(name="xp", bufs=bufs_x) as xpool, \
             tc.tile_pool(name="hp", bufs=bufs_h) as hpool, \
             tc.tile_pool(name="gp", bufs=bufs_g) as gpool, \
             tc.tile_pool(name="tp", bufs=bufs_tmp) as tpool, \
             tc.tile_pool(name="us", bufs=bufs_us) as uspool, \
             tc.tile_pool(name="ps", bufs=8, space="PSUM") as pspool:
            w8_sb = cpool.tile([128, KD, G], F8, name="w8_sb")
            nc.sync.dma_start(out=w8_sb[:], in_=w8_d[:])
            u8_sb = cpool.tile([128, KH, G], F8, name="u8_sb")
            wb_sb = cpool.tile([128, KD * G], BF16, name="wb_sb")

            def load_big():
                nc.sync.dma_start(out=u8_sb[:], in_=u8_d[:])
                nc.sync.dma_start(out=wb_sb[:], in_=wb_d[:])
            if reps != 1:
                load_big()
            b_sb = cpool.tile([128, NGT], F32, name="b_sb")
            nc.sync.dma_start(out=b_sb[:], in_=b_d[:])
            # c state, bf16; final h goes to the separate f32 out tile
            c_sb = cpool.tile([128, KH * BL], BF16, name="c_sb")
            o32_sb = cpool.tile([128, KH * BL], F32, name="o32_sb")

            MULT = mybir.AluOpType.mult
            SUB = mybir.AluOpType.subtract

            h_fake8 = h_fakeb = None
            if nodep:
                # timing-diagnostic mode: h matmuls read a constant tile
                # instead of the previous step's h (breaks the recurrence
                # dependency; numerics intentionally wrong)
                h_fake8 = cpool.tile([128, KH, BL], F8, name="h_fake8")
                nc.sync.dma_start(out=h_fake8[:],
                                  in_=u8_d[:, 0:KH * BL])
                h_fakeb = cpool.tile([128, KH, BL], BF16, name="h_fakeb")
                nc.sync.dma_start(out=h_fakeb[:], in_=ubs_d[:, 0:KH * BL])

            def body(rep):
                h_prev = None
                for t in range(TS):
                    xp = xprec(t)
                    x_t = xpool.tile([128, KD, BL], F8 if xp == '8' else BF16,
                                     name=f"x_{rep}_{t}", tag="x")
                    src = x8_d if xp == '8' else xb_d
                    for kd in range(KD):
                        nc.sync.dma_start(
                            out=x_t[:, kd, :],
                            in_=src[t * D + kd * 128: t * D + (kd + 1) * 128, :])
                    if t == 0 and reps == 1:
                        load_big()   # t=0 matmuls need only w8 + x_0
                    if t < TS - 1:
                        h_new = hpool.tile(
                            [128, KH, BL],
                            F8 if hprec(t + 1) == '8' else BF16,
                            name=f"h_{rep}_{t}", tag="h")
                    else:
                        h_new = None
                    hp = hprec(t)
                    # software-pipelined per-slice work: the c-update tail
                    # (t1/add) runs one slice late and the h-production
                    # (tanh(c) + o*tanh) runs ~3 slices late, so no engine
                    # queue head ever waits on a result produced just
                    # before it.
                    slices = {}

                    def c_tail(hj):
                        # finalize c for slice hj (t>0 only)
                        gfj, _goj, t1j, t2j = slices[hj]
                        csj = c_sb[:, hj * BL:(hj + 1) * BL]
                        nc.vector.tensor_mul(t1j, gfj, csj)
                        nc.vector.tensor_add(csj, t1j, t2j)

                    def h_pair(k):
                        # h = o*tanh(c) for slices 2k, 2k+1
                        s2 = tpool.tile([128, 2 * BL], BF16,
                                        name=f"s2_{rep}_{t}_{k}",
                                        tag="s2", bufs=2)
                        nc.scalar.activation(
                            s2[:], c_sb[:, 2 * k * BL:(2 * k + 2) * BL],
                            AF.Tanh)
                        for j in range(2):
                            hj = 2 * k + j
                            goj = slices[hj][1]
                            hdst = (h_new[:, hj, :] if t < TS - 1
                                    else o32_sb[:, hj * BL:(hj + 1) * BL])
                            nc.vector.tensor_mul(
                                hdst, goj, s2[:, j * BL:(j + 1) * BL])

                    for ht in range(KH):
                        gates = gpool.tile([128, 4 * BL], BF16,
                                           name=f"gates_{rep}_{t}_{ht}",
                                           tag="g")
                        skip0 = 1 if t == 0 else 0  # f gate unused at t=0
                        pss = [(pspool.tile([128, BL], F32,
                                            name=f"ps_{rep}_{t}_{gi * KH + ht}",
                                            tag="ps")
                                if gi >= skip0 else None)
                               for gi in range(4)]

                        def mm_group(gis):
                            for gi in gis:
                                gt = gi * KH + ht
                                if xp == '8':
                                    for k2 in range(KD // 2):
                                        nc.tensor.matmul(
                                            pss[gi][:],
                                            w8_sb[:, 2 * k2:2 * k2 + 2,
                                                  gt * 128:(gt + 1) * 128],
                                            x_t[:, 2 * k2:2 * k2 + 2, :],
                                            start=(k2 == 0),
                                            stop=(t == 0 and
                                                  k2 == KD // 2 - 1),
                                            perf_mode=DR)
                                else:
                                    for kd in range(KD):
                                        nc.tensor.matmul(
                                            pss[gi][:],
                                            wb_sb[:, kd * G + gt * 128:
                                                  kd * G + gt * 128 + 128],
                                            x_t[:, kd, :],
                                            start=(kd == 0),
                                            stop=(t == 0 and kd == KD - 1))
                            if t > 0:
                                if hp == '8':
                                    # kh-major across gates: the freshest
                                    # h slices of the previous step are
                                    # consumed last.
                                    for k2 in range(KH // 2):
                                        for gi in gis:
                                            gt = gi * KH + ht
                                            nc.tensor.matmul(
                                                pss[gi][:],
                                                u8_sb[:, 2 * k2:2 * k2 + 2,
                                                      gt * 128:
                                                      (gt + 1) * 128],
                                                h_prev[:, 2 * k2:2 * k2 + 2,
                                                       :],
                                                start=False,
                                                stop=(k2 == KH // 2 - 1),
                                                perf_mode=DR)
                                else:
                                    uts = {}
                                    for gi in gis:
                                        gt = gi * KH + ht
                                        ut = uspool.tile(
                                            [128, KH, 128], BF16,
                                            name=f"us_{rep}_{t}_{gt}",
                                            tag="us")
                                        nc.sync.dma_start(
                                            out=ut[:],
                                            in_=ubs_d[:, gt * KH * 128:
                                                      (gt + 1) * KH * 128])
                                        uts[gi] = ut
                                    for kh in range(KH):
                                        for gi in gis:
                                            nc.tensor.matmul(
                                                pss[gi][:],
                                                uts[gi][:, kh, :],
                                                h_prev[:, kh, :],
                                                start=False,
                                                stop=(kh == KH - 1))

                        # i and the c-gate feed the DVE chain first: their
                        # matmul groups and activations go first, read
                        # directly from PSUM as truncated bf16 halfwords
                        # (the f32 PSUM port runs at half rate).  f and o
                        # drain through a DVE psum->sbuf copy + SBUF-side
                        # activation to split the PSUM port load across
                        # both engines.
                        mm_group([1, 3])
                        mm_group([0, 2] if t > 0 else [2])
                        for gi in (1, 3):
                            gt = gi * KH + ht
                            nc.scalar.activation(
                                gates[:, gi * BL:(gi + 1) * BL],
                                pss[gi][:].bitcast(BF16)[:, 1::2],
                                AF.Tanh if gi == 3 else AF.Sigmoid,
                                bias=b_sb[:, gt:gt + 1])
                        pz = tpool.tile([128, 2 * BL], BF16,
                                        name=f"pz_{rep}_{t}_{ht}", tag="pz",
                                        bufs=3)
                        for j, gi in enumerate((0, 2)):
                            if gi < skip0:
                                continue
                            gt = gi * KH + ht
                            nc.vector.tensor_copy(
                                pz[:, j * BL:(j + 1) * BL], pss[gi][:])
                            nc.scalar.activation(
                                gates[:, gi * BL:(gi + 1) * BL],
                                pz[:, j * BL:(j + 1) * BL],
                                AF.Sigmoid, bias=b_sb[:, gt:gt + 1])
                        gf = gates[:, 0 * BL:1 * BL]
                        gi_ = gates[:, 1 * BL:2 * BL]
                        go = gates[:, 2 * BL:3 * BL]
                        gc = gates[:, 3 * BL:4 * BL]
                        cs = c_sb[:, ht * BL:(ht + 1) * BL]
                        tmp = tpool.tile([128, 2 * BL], BF16,
                                         name=f"tmp_{rep}_{t}_{ht}",
                                         tag="tmp", bufs=4)
                        t1 = tmp[:, 0 * BL:1 * BL]
                        t2 = tmp[:, 1 * BL:2 * BL]
                        slices[ht] = (gf, go, t1, t2)
                        # partial c-update now; the f*c tail one slice
                        # later (act_f flows through the copy path and
                        # would head-block the DVE queue otherwise)
                        if t == 0:
                            nc.vector.tensor_mul(cs, gi_, gc)
                        else:
                            nc.vector.tensor_mul(t2, gi_, gc)
                            if ht >= 1:
                                c_tail(ht - 1)
                        # h-production (tanh(c) + o*tanh) runs ~3 slices
                        # late; next step consumes h pairs kh-major in the
                        # same order they finish here.
                        if ht >= 5 - 2 * skip0 and ht % 2 == 1:
                            h_pair((ht - 5 + 2 * skip0) // 2)
                    if t > 0:
                        c_tail(KH - 1)
                        h_pair(2)
                        h_pair(3)
                    else:
                        h_pair(3)
                    if nodep:
                        h_prev = (h_fake8 if t + 1 < TS and
                                  hprec(t + 1) == '8' else h_fakeb)
                    else:
                        h_prev = h_new
                for kh in range(KH):
                    nc.sync.dma_start(out=out_d[kh * 128:(kh + 1) * 128, :],
                                      in_=o32_sb[:, kh * BL:(kh + 1) * BL])

            if reps == 1:
                body(0)
            else:
                with tc.For_i(0, reps, 1):
                    body(0)
    return nc


_BUILT = None


def _get_built():
    global _BUILT
    if _BUILT is None:
        nc = bacc.Bacc("TRN2", num_devices=NCORES)
        build_lstm(nc)
        nc.compile()
        _BUILT = nc
    return _BUILT


def _prep_inputs(x, wf, wi, wo, wc, uf, ui, uo, uc, bf, bi, bo, bc):
    bf16 = ml_dtypes.bfloat16
    e4 = ml_dtypes.float8_e4m3
    W = np.concatenate([wf, wi, wo, wc], axis=1)                   # [D, 4H]
    U = np.concatenate([uf, ui, uo, uc], axis=1)                   # [H, 4H]
    b = np.concatenate([bf, bi, bo, bc], axis=1).astype(np.float32)
    b_t = np.ascontiguousarray(b.reshape(NGT, 128).T)              # [128, NGT]
    # weight images: (p, k, g) = W[k*128+p, g], flattened per partition
    w8 = np.ascontiguousarray(
        W.reshape(KD, 128, G).transpose(1, 0, 2).reshape(128, KD * G)
    ).astype(e4)
    u8 = np.ascontiguousarray(
        U.reshape(KH, 128, G).transpose(1, 0, 2).reshape(128, KH * G)
    ).astype(e4)
    wb = np.ascontiguousarray(
        W.reshape(KD, 128, G).transpose(1, 0, 2).reshape(128, KD * G)
    ).astype(bf16)
    # streamed bf16 U, gt-major: (p, gt, kh, m) = U[kh*128+p, gt*128+m]
    ubs = np.ascontiguousarray(
        U.reshape(KH, 128, NGT, 128).transpose(1, 2, 0, 3).reshape(128, -1)
    ).astype(bf16)
    # x [B, T, D] -> per-core [T*D, BL] with xT[t*D+d, b] = x[b, t, d]
    xt = np.ascontiguousarray(np.transpose(x, (1, 2, 0)))  # [T, D, B] f32
    in_maps = []
    for c in range(NCORES):
        xc = np.ascontiguousarray(
            xt[:, :, c * BL:(c + 1) * BL].reshape(T * D, BL))
        in_maps.append({"x8": xc.astype(e4), "xb": xc.astype(bf16),
                        "w8": w8, "u8": u8, "wb": wb, "ubs": ubs,
                        "b": b_t})
    return in_maps


def kernel(x, wf, wi, wo, wc, uf, ui, uo, uc, bf, bi, bo, bc):
    nc = _get_built()
    in_maps = _prep_inputs(x, wf, wi, wo, wc, uf, ui, uo, uc, bf, bi, bo, bc)
    res = run_bass_kernel_spmd(nc, in_maps, list(range(NCORES)))
    out = np.empty((B, H), np.float32)
    for c in range(NCORES):
        out[c * BL:(c + 1) * BL, :] = res.results[c]["h_out"].T
    return out


# revision 21
# speedup vs baseline: 1.0301x; 1.0301x over previous
"""Trainium2 Bass kernel for the CustomLSTM problem.

Contract: kernel(**inputs) takes the FULL unsharded numpy inputs
(x [4096,16,512] f32, per-gate weights/biases) and returns the FULL
output h_last [4096, 1024] f32.

Strategy (data-parallel over 8 NeuronCores):
  - shard batch B=4096 -> 512 per core; replicate weights.
  - per core, per timestep t, compute fused gates in transposed layout
    gT [4H=4096, B=512] as one PSUM accumulation per 128-row gate tile.
  - every matmul is an fp8e4m3 MatmulPerfMode.DoubleRow instruction
    (2x128 contraction at 0.5 cycles/row -> ~4x bf16 throughput).
    Early steps run single-pass fp8; the last TAILX steps add
    residual-compensation passes because LSTM errors injected early
    decay through the forget gates while late-step errors hit the
    output directly:
      x-part (last TAILX steps):  x16@W8 + xl16@W8 + x1@Wl16
      h-part (last TAILH steps):  h16@U8 + dh16@U8 + h1@Ul16
    where v16 = fp8(16 v), vl16/dh16 = fp8(16 v - 16 fp8(v)) and
    Wl16 = fp8(16 (W - fp8(W))).  All terms accumulate at a uniform
    16x scale in one PSUM group (power-of-two scaling is exact in
    fp8), and the activation's scale operand applies the 1/16.
  - gate nonlinearities: i and the c-gate (real Tanh; it shares an
    activation table set with Sigmoid) read PSUM directly as truncated
    bf16 halfwords (the f32 PSUM port runs at half rate); f and o
    drain through a DVE psum->sbuf copy + SBUF-side activation to
    split the PSUM port load across both engines.
  - element-wise chain in bf16 (2x DVE mode): t2=i*ch, t1=f*c,
    c=t1+t2, h-production from o*tanh(c); the c update tail runs one
    slice late and h-production ~3 slices late so no engine queue
    head waits on a just-produced result.
  - c stays bf16 in SBUF; the final step writes h into an f32 tile
    that is DMA'd out.
"""

import numpy as np
import ml_dtypes

import concourse.bacc as bacc
import concourse.mybir as mybir
from concourse.tile import TileContext
from concourse.bass_utils import run_bass_kernel_spmd

F32 = mybir.dt.float32
BF16 = mybir.dt.bfloat16
F8 = mybir.dt.float8e4
AF = mybir.ActivationFunctionType
DR = mybir.MatmulPerfMode.DoubleRow

B, T, D, H = 4096, 16, 512, 1024
NCORES = 8
BL = B // NCORES          # batch per core
G = 4 * H                 # fused gate dim
KD = D // 128             # x contraction tiles
KH = H // 128             # h contraction tiles
NGT = G // 128            # gate tiles

TAILX = 5                 # last TAILX steps: compensated 3-term x-part
TAILH = 2                 # last TAILH steps: compensated 3-term h-part


def build_lstm(nc, reps=1, tailx=TAILX, tailh=TAILH, bufs_g=8, bufs_x=2,
               bufs_h=2, bufs_tmp=3, t_steps=None, nodep=False):
    TS = t_steps if t_steps is not None else T
    x8_d = nc.declare_dram_parameter("x8", [T * D, BL], F8, isOutput=False)
    x16_d = nc.declare_dram_parameter("x16", [T * D, BL], F8, isOutput=False)
    xl16_d = nc.declare_dram_parameter("xl16", [T * D, BL], F8, isOutput=False)
    w8_d = nc.declare_dram_parameter("w8", [128, KD * G], F8, isOutput=False)
    u8_d = nc.declare_dram_parameter("u8", [128, KH * G], F8, isOutput=False)
    wl_d = nc.declare_dram_parameter("wl", [128, KD * G], F8, isOutput=False)
    ul_d = nc.declare_dram_parameter("ul", [128, KH * G], F8, isOutput=False)
    b_d = nc.declare_dram_parameter("b", [128, NGT], F32, isOutput=False)
    out_d = nc.declare_dram_parameter("h_out", [H, BL], F32, isOutput=True)

    def xmode(t):
        return 'comp' if t >= TS - tailx else '1p'

    def hmode(t):
        # mode of the h-projection performed AT step t (t >= 1)
        return 'comp' if t >= TS - tailh else '1p'

    def hprod(t):
        # what step t must produce for step t+1's h-projection
        if t + 1 >= TS:
            return 'out'
        if hmode(t + 1) == 'comp':
            return 'h3'       # h16, dh16, h1
        if xmode(t + 1) == 'comp':
            return 'h16'      # scaled single-pass
        return 'h8'           # unscaled single-pass

    with TileContext(nc) as tc:
        with tc.tile_pool(name="const", bufs=1) as cpool, \
             tc.tile_pool(name="xp", bufs=bufs_x) as xpool, \
             tc.tile_pool(name="hp", bufs=bufs_h) as hpool, \
             tc.tile_pool(name="gp", bufs=bufs_g) as gpool, \
             tc.tile_pool(name="tp", bufs=bufs_tmp) as tpool, \
             tc.tile_pool(name="ps", bufs=8, space="PSUM") as pspool:
            w8_sb = cpool.tile([128, KD, G], F8, name="w8_sb")
            nc.sync.dma_start(out=w8_sb[:], in_=w8_d[:])
            u8_sb = cpool.tile([128, KH, G], F8, name="u8_sb")
            wl_sb = cpool.tile([128, KD, G], F8, name="wl_sb")
            ul_sb = cpool.tile([128, KH, G], F8, name="ul_sb")

            def load_big():
                nc.sync.dma_start(out=u8_sb[:], in_=u8_d[:])
                nc.sync.dma_start(out=wl_sb[:], in_=wl_d[:])
                nc.sync.dma_start(out=ul_sb[:], in_=ul_d[:])
            if reps != 1:
                load_big()
            b_sb = cpool.tile([128, NGT], F32, name="b_sb")
            nc.sync.dma_start(out=b_sb[:], in_=b_d[:])
            # c state, bf16; the final step's h pairs stream out through
            # small rotating f32 tiles
            c_sb = cpool.tile([128, KH * BL], BF16, name="c_sb")

            MULT = mybir.AluOpType.mult
            SUB = mybir.AluOpType.subtract

            h_fake1 = h_fake3 = None
            if nodep:
                # timing-diagnostic mode: h matmuls read a constant tile
                # instead of the previous step's h (numerics wrong)
                h_fake1 = cpool.tile([128, 1, KH, BL], F8, name="h_fake1")
                nc.sync.dma_start(out=h_fake1[:], in_=u8_d[:, 0:KH * BL])
                h_fake3 = cpool.tile([128, 3, KH, BL], F8, name="h_fake3")
                nc.sync.dma_start(out=h_fake3[:], in_=u8_d[:, 0:3 * KH * BL])

            def body(rep):
                h_prev = None
                for t in range(TS):
                    xm = xmode(t)
                    nx = 1 if xm == '1p' else 3
                    x_t = xpool.tile([128, nx, KD, BL], F8,
                                     name=f"x_{rep}_{t}", tag="x")
                    srcs = ([x8_d] if xm == '1p'
                            else [x16_d, xl16_d, x8_d])
                    for j, src in enumerate(srcs):
                        for kd in range(KD):
                            nc.sync.dma_start(
                                out=x_t[:, j, kd, :],
                                in_=src[t * D + kd * 128:
                                        t * D + (kd + 1) * 128, :])
                    if t == 0 and reps == 1:
                        load_big()   # t=0 matmuls need only w8 + x_0
                    hp_mode = hprod(t)
                    if hp_mode == 'out':
                        h_new = None
                    elif hp_mode == 'h3':
                        h_new = hpool.tile([128, 3, KH, BL], F8,
                                           name=f"h_{rep}_{t}", tag="h")
                    else:
                        h_new = hpool.tile([128, 1, KH, BL], F8,
                                           name=f"h_{rep}_{t}", tag="h")
                    hm = hmode(t)
                    scale = 1.0 if xm == '1p' else 1.0 / 16.0
                    slices = {}

                    def c_tail(hj):
                        gfj, _goj, t1j, t2j = slices[hj]
                        csj = c_sb[:, hj * BL:(hj + 1) * BL]
                        nc.vector.tensor_mul(t1j, gfj, csj)
                        nc.vector.tensor_add(csj, t1j, t2j)

                    def h_pair(k):
                        # produce h for slices 2k, 2k+1 from o and tanh(c)
                        s2 = tpool.tile([128, 2 * BL], BF16,
                                        name=f"s2_{rep}_{t}_{k}",
                                        tag="s2", bufs=2)
                        nc.scalar.activation(
                            s2[:], c_sb[:, 2 * k * BL:(2 * k + 2) * BL],
                            AF.Tanh)
                        for j in range(2):
                            hj = 2 * k + j
                            goj = slices[hj][1]
                            s2j = s2[:, j * BL:(j + 1) * BL]
                            if hp_mode == 'out':
                                o32 = tpool.tile(
                                    [128, BL], F32,
                                    name=f"o32_{rep}_{t}_{hj}",
                                    tag="o32", bufs=2)
                                nc.vector.tensor_mul(o32[:], goj, s2j)
                                nc.sync.dma_start(
                                    out=out_d[hj * 128:(hj + 1) * 128, :],
                                    in_=o32[:])
                            elif hp_mode == 'h8':
                                nc.vector.tensor_mul(
                                    h_new[:, 0, hj, :], goj, s2j)
                            elif hp_mode == 'h16':
                                nc.vector.scalar_tensor_tensor(
                                    h_new[:, 0, hj, :], goj, 16.0, s2j,
                                    MULT, MULT)
                            else:  # h3: h16, dh16, h1
                                hf = tpool.tile([128, BL], F32,
                                                name=f"hf_{rep}_{t}_{hj}",
                                                tag="hf", bufs=2)
                                nc.vector.scalar_tensor_tensor(
                                    hf[:], goj, 16.0, s2j, MULT, MULT)
                                nc.vector.tensor_copy(
                                    h_new[:, 0, hj, :], hf[:])
                                nc.vector.tensor_sub(
                                    h_new[:, 1, hj, :], hf[:],
                                    h_new[:, 0, hj, :])
                                nc.vector.tensor_mul(
                                    h_new[:, 2, hj, :], goj, s2j)

                    for ht in range(KH):
                        gates = gpool.tile([128, 4 * BL], BF16,
                                           name=f"gates_{rep}_{t}_{ht}",
                                           tag="g")
                        skip0 = 1 if t == 0 else 0  # f gate unused at t=0
                        pss = [(pspool.tile([128, BL], F32,
                                            name=f"ps_{rep}_{t}_{gi * KH + ht}",
                                            tag="ps")
                                if gi >= skip0 else None)
                               for gi in range(4)]

                        def mm_group(gis):
                            for gi in gis:
                                gt = gi * KH + ht
                                first = True
                                # x terms: (x slot, weight tile)
                                xterms = ([(0, w8_sb)] if xm == '1p'
                                          else [(0, w8_sb), (1, w8_sb),
                                                (2, wl_sb)])
                                for slot, wsb in xterms:
                                    for k2 in range(KD // 2):
                                        nc.tensor.matmul(
                                            pss[gi][:],
                                            wsb[:, 2 * k2:2 * k2 + 2,
                                                gt * 128:(gt + 1) * 128],
                                            x_t[:, slot,
                                                2 * k2:2 * k2 + 2, :],
                                            start=first,
                                            stop=(t == 0 and
                                                  slot == xterms[-1][0] and
                                                  k2 == KD // 2 - 1),
                                            perf_mode=DR)
                                        first = False
                            if t > 0:
                                hterms = ([(0, u8_sb)] if hm == '1p'
                                          else [(0, u8_sb), (1, u8_sb),
                                                (2, ul_sb)])
                                # kh-major across gates: the freshest h
                                # slices of the previous step are consumed
                                # last.
                                for slot, usb in hterms:
                                    for k2 in range(KH // 2):
                                        for gi in gis:
                                            gt = gi * KH + ht
                                            nc.tensor.matmul(
                                                pss[gi][:],
                                                usb[:, 2 * k2:2 * k2 + 2,
                                                    gt * 128:
                                                    (gt + 1) * 128],
                                                h_prev[:, slot,
                                                       2 * k2:2 * k2 + 2,
                                                       :],
                                                start=False,
                                                stop=(slot ==
                                                      hterms[-1][0] and
                                                      k2 == KH // 2 - 1),
                                                perf_mode=DR)

                        mm_group([1, 3])
                        mm_group([0, 2] if t > 0 else [2])
                        for gi in (1, 3):
                            gt = gi * KH + ht
                            nc.scalar.activation(
                                gates[:, gi * BL:(gi + 1) * BL],
                                pss[gi][:].bitcast(BF16)[:, 1::2],
                                AF.Tanh if gi == 3 else AF.Sigmoid,
                                bias=b_sb[:, gt:gt + 1], scale=scale)
                        pz = tpool.tile([128, 2 * BL], BF16,
                                        name=f"pz_{rep}_{t}_{ht}", tag="pz",
                                        bufs=3)
                        for j, gi in enumerate((0, 2)):
                            if gi < skip0:
                                continue
                            gt = gi * KH + ht
                            nc.vector.tensor_copy(
                                pz[:, j * BL:(j + 1) * BL], pss[gi][:])
                            nc.scalar.activation(
                                gates[:, gi * BL:(gi + 1) * BL],
                                pz[:, j * BL:(j + 1) * BL],
                                AF.Sigmoid, bias=b_sb[:, gt:gt + 1],
                                scale=scale)
                        gf = gates[:, 0 * BL:1 * BL]
                        gi_ = gates[:, 1 * BL:2 * BL]
                        go = gates[:, 2 * BL:3 * BL]
                        gc = gates[:, 3 * BL:4 * BL]
                        cs = c_sb[:, ht * BL:(ht + 1) * BL]
                        tmp = tpool.tile([128, 2 * BL], BF16,
                                         name=f"tmp_{rep}_{t}_{ht}",
                                         tag="tmp", bufs=4)
                        t1 = tmp[:, 0 * BL:1 * BL]
                        t2 = tmp[:, 1 * BL:2 * BL]
                        slices[ht] = (gf, go, t1, t2)
                        # partial c-update now; the f*c tail one slice
                        # later (act_f flows through the copy path and
                        # would head-block the DVE queue otherwise)
                        if t == 0:
                            nc.vector.tensor_mul(cs, gi_, gc)
                        else:
                            nc.vector.tensor_mul(t2, gi_, gc)
                            if ht >= 1:
                                c_tail(ht - 1)
                        # h-production runs ~3 slices late; the next step
                        # consumes h pairs kh-major in the same order
                        # they finish here.
                        if ht >= 5 - 2 * skip0 and ht % 2 == 1:
                            h_pair((ht - 5 + 2 * skip0) // 2)
                    if t > 0:
                        c_tail(KH - 1)
                        h_pair(2)
                        h_pair(3)
                    else:
                        h_pair(3)
                    if nodep:
                        h_prev = h_fake3 if hprod(t) == 'h3' else h_fake1
                    else:
                        h_prev = h_new

            if reps == 1:
                body(0)
            else:
                with tc.For_i(0, reps, 1):
                    body(0)
    return nc


_BUILT = None


def _get_built():
    global _BUILT
    if _BUILT is None:
        nc = bacc.Bacc("TRN2", num_devices=NCORES)
        build_lstm(nc)
        nc.compile()
        _BUILT = nc
    return _BUILT


def _prep_inputs(x, wf, wi, wo, wc, uf, ui, uo, uc, bf, bi, bo, bc):
    e4 = ml_dtypes.float8_e4m3
    W = np.concatenate([wf, wi, wo, wc], axis=1)                   # [D, 4H]
    U = np.concatenate([uf, ui, uo, uc], axis=1)                   # [H, 4H]
    b = np.concatenate([bf, bi, bo, bc], axis=1).astype(np.float32)
    b_t = np.ascontiguousarray(b.reshape(NGT, 128).T)              # [128, NGT]

    def wimg(M, k, scale=1.0):
        return np.ascontiguousarray(
            (M * scale).reshape(k, 128, G).transpose(1, 0, 2)
            .reshape(128, k * G)).astype(e4)
    w8 = wimg(W, KD)
    u8 = wimg(U, KH)
    wl = wimg(16.0 * (W - w8.astype(np.float32)
                      .reshape(128, KD, G).transpose(1, 0, 2)
                      .reshape(D, G)), KD)
    ul = wimg(16.0 * (U - u8.astype(np.float32)
                      .reshape(128, KH, G).transpose(1, 0, 2)
                      .reshape(H, G)), KH)
    # x [B, T, D] -> per-core [T*D, BL] with xT[t*D+d, b] = x[b, t, d]
    xt = np.ascontiguousarray(np.transpose(x, (1, 2, 0)))  # [T, D, B] f32
    in_maps = []
    for c in range(NCORES):
        xc = np.ascontiguousarray(
            xt[:, :, c * BL:(c + 1) * BL].reshape(T * D, BL))
        x8 = xc.astype(e4)
        x16 = (xc * 16.0).astype(e4)
        xl16 = (xc * 16.0 - x16.astype(np.float32)).astype(e4)
        in_maps.append({"x8": x8, "x16": x16, "xl16": xl16,
                        "w8": w8, "u8": u8, "wl": wl, "ul": ul,
                        "b": b_t})
    return in_maps


def kernel(x, wf, wi, wo, wc, uf, ui, uo, uc, bf, bi, bo, bc):
    nc = _get_built()
    in_maps = _prep_inputs(x, wf, wi, wo, wc, uf, ui, uo, uc, bf, bi, bo, bc)
    res = run_bass_kernel_spmd(nc, in_maps, list(range(NCORES)))
    out = np.empty((B, H), np.float32)
    for c in range(NCORES):
        out[c * BL:(c + 1) * BL, :] = res.results[c]["h_out"].T
    return out


# revision 22
# speedup vs baseline: 1.3571x; 1.3174x over previous
"""Trainium2 Bass kernel for the CustomLSTM problem.

Contract: kernel(**inputs) takes the FULL unsharded numpy inputs
(x [4096,16,512] f32, per-gate weights/biases) and returns the FULL
output h_last [4096, 1024] f32.

Strategy (data-parallel over 8 NeuronCores):
  - shard batch B=4096 -> 512 per core; replicate weights.
  - per core, per timestep t, compute fused gates in transposed layout
    gT [4H=4096, B=512] as one PSUM accumulation per 128-row gate tile.
  - every matmul is an fp8e4m3 MatmulPerfMode.DoubleRow instruction
    (2x128 contraction at 0.5 cycles/row -> ~4x bf16 throughput).
    Early steps run single-pass fp8; the last TAILX steps add
    residual-compensation passes because LSTM errors injected early
    decay through the forget gates while late-step errors hit the
    output directly:
      x-part (last TAILX steps):  x16@W8 + xl16@W8 + x1@Wl16
      h-part (last TAILH steps):  h16@U8 + dh16@U8 + h1@Ul16
    where v16 = fp8(16 v), vl16/dh16 = fp8(16 v - 16 fp8(v)) and
    Wl16 = fp8(16 (W - fp8(W))).  All terms accumulate at a uniform
    16x scale in one PSUM group (power-of-two scaling is exact in
    fp8), and the activation's scale operand applies the 1/16.
  - gate nonlinearities: i and the c-gate (real Tanh; it shares an
    activation table set with Sigmoid) read PSUM directly as truncated
    bf16 halfwords (the f32 PSUM port runs at half rate); f and o
    drain through a DVE psum->sbuf copy + SBUF-side activation to
    split the PSUM port load across both engines.
  - element-wise chain in bf16 (2x DVE mode): t2=i*ch, t1=f*c,
    c=t1+t2, h-production from o*tanh(c); the c update tail runs one
    slice late and h-production ~3 slices late so no engine queue
    head waits on a just-produced result.
  - c stays bf16 in SBUF; the final step writes h into an f32 tile
    that is DMA'd out.
"""

import numpy as np
import ml_dtypes

import concourse.bacc as bacc
import concourse.mybir as mybir
from concourse.tile import TileContext
from concourse.bass_utils import run_bass_kernel_spmd

F32 = mybir.dt.float32
BF16 = mybir.dt.bfloat16
F8 = mybir.dt.float8e4
AF = mybir.ActivationFunctionType
DR = mybir.MatmulPerfMode.DoubleRowSwInterleave

B, T, D, H = 4096, 16, 512, 1024
NCORES = 8
BL = B // NCORES          # batch per core
G = 4 * H                 # fused gate dim
KD = D // 128             # x contraction tiles
KH = H // 128             # h contraction tiles
NGT = G // 128            # gate tiles

TAILX = 5                 # last TAILX steps: compensated 3-term x-part
TAILH = 2                 # last TAILH steps: compensated 3-term h-part


def build_lstm(nc, reps=1, tailx=TAILX, tailh=TAILH, bufs_g=8, bufs_x=2,
               bufs_h=2, bufs_tmp=3, t_steps=None, nodep=False):
    TS = t_steps if t_steps is not None else T
    x8_d = nc.declare_dram_parameter("x8", [T * D, BL], F8, isOutput=False)
    x16_d = nc.declare_dram_parameter("x16", [T * D, BL], F8, isOutput=False)
    xl16_d = nc.declare_dram_parameter("xl16", [T * D, BL], F8, isOutput=False)
    w8_d = nc.declare_dram_parameter("w8", [128, KD * G], F8, isOutput=False)
    u8_d = nc.declare_dram_parameter("u8", [128, KH * G], F8, isOutput=False)
    wl_d = nc.declare_dram_parameter("wl", [128, KD * G], F8, isOutput=False)
    ul_d = nc.declare_dram_parameter("ul", [128, KH * G], F8, isOutput=False)
    b_d = nc.declare_dram_parameter("b", [128, NGT], F32, isOutput=False)
    out_d = nc.declare_dram_parameter("h_out", [H, BL], F32, isOutput=True)

    def xmode(t):
        return 'comp' if t >= TS - tailx else '1p'

    def hmode(t):
        # mode of the h-projection performed AT step t (t >= 1)
        return 'comp' if t >= TS - tailh else '1p'

    def hprod(t):
        # what step t must produce for step t+1's h-projection
        if t + 1 >= TS:
            return 'out'
        if hmode(t + 1) == 'comp':
            return 'h3'       # h16, dh16, h1
        if xmode(t + 1) == 'comp':
            return 'h16'      # scaled single-pass
        return 'h8'           # unscaled single-pass

    with TileContext(nc) as tc:
        with tc.tile_pool(name="const", bufs=1) as cpool, \
             tc.tile_pool(name="xp", bufs=bufs_x) as xpool, \
             tc.tile_pool(name="hp", bufs=bufs_h) as hpool, \
             tc.tile_pool(name="gp", bufs=bufs_g) as gpool, \
             tc.tile_pool(name="tp", bufs=bufs_tmp) as tpool, \
             tc.tile_pool(name="ps", bufs=8, space="PSUM") as pspool:
            w8_sb = cpool.tile([128, KD // 2, 2 * G], F8, name="w8_sb")
            nc.sync.dma_start(out=w8_sb[:], in_=w8_d[:])
            u8_sb = cpool.tile([128, KH // 2, 2 * G], F8, name="u8_sb")
            wl_sb = cpool.tile([128, KD // 2, 2 * G], F8, name="wl_sb")
            ul_sb = cpool.tile([128, KH // 2, 2 * G], F8, name="ul_sb")

            def load_big():
                nc.sync.dma_start(out=u8_sb[:], in_=u8_d[:])
                nc.sync.dma_start(out=wl_sb[:], in_=wl_d[:])
                nc.sync.dma_start(out=ul_sb[:], in_=ul_d[:])
            if reps != 1:
                load_big()
            b_sb = cpool.tile([128, NGT], F32, name="b_sb")
            nc.sync.dma_start(out=b_sb[:], in_=b_d[:])
            # c state, bf16; the final step's h pairs stream out through
            # small rotating f32 tiles
            c_sb = cpool.tile([128, KH * BL], BF16, name="c_sb")

            MULT = mybir.AluOpType.mult
            SUB = mybir.AluOpType.subtract

            h_fake1 = h_fake3 = None
            if nodep:
                # timing-diagnostic mode: h matmuls read a constant tile
                # instead of the previous step's h (numerics wrong)
                h_fake1 = cpool.tile([128, 1, KH, BL], F8, name="h_fake1")
                nc.sync.dma_start(out=h_fake1[:], in_=u8_d[:, 0:KH * BL])
                h_fake3 = cpool.tile([128, 3, KH, BL], F8, name="h_fake3")
                nc.sync.dma_start(out=h_fake3[:], in_=u8_d[:, 0:3 * KH * BL])

            def body(rep):
                h_prev = None
                for t in range(TS):
                    xm = xmode(t)
                    nx = 1 if xm == '1p' else 3
                    x_t = xpool.tile([128, nx, KD, BL], F8,
                                     name=f"x_{rep}_{t}", tag="x")
                    srcs = ([x8_d] if xm == '1p'
                            else [x16_d, xl16_d, x8_d])
                    for j, src in enumerate(srcs):
                        for kd in range(KD):
                            nc.sync.dma_start(
                                out=x_t[:, j, kd, :],
                                in_=src[t * D + kd * 128:
                                        t * D + (kd + 1) * 128, :])
                    if t == 0 and reps == 1:
                        load_big()   # t=0 matmuls need only w8 + x_0
                    hp_mode = hprod(t)
                    if hp_mode == 'out':
                        h_new = None
                    elif hp_mode == 'h3':
                        h_new = hpool.tile([128, 3, KH, BL], F8,
                                           name=f"h_{rep}_{t}", tag="h")
                    else:
                        h_new = hpool.tile([128, 1, KH, BL], F8,
                                           name=f"h_{rep}_{t}", tag="h")
                    hm = hmode(t)
                    scale = 1.0 if xm == '1p' else 1.0 / 16.0
                    slices = {}

                    def c_tail(hj):
                        gfj, _goj, t1j, t2j = slices[hj]
                        csj = c_sb[:, hj * BL:(hj + 1) * BL]
                        nc.vector.tensor_mul(t1j, gfj, csj)
                        nc.vector.tensor_add(csj, t1j, t2j)

                    def h_pair(k):
                        # produce h for slices 2k, 2k+1 from o and tanh(c)
                        s2 = tpool.tile([128, 2 * BL], BF16,
                                        name=f"s2_{rep}_{t}_{k}",
                                        tag="s2", bufs=2)
                        nc.scalar.activation(
                            s2[:], c_sb[:, 2 * k * BL:(2 * k + 2) * BL],
                            AF.Tanh)
                        for j in range(2):
                            hj = 2 * k + j
                            goj = slices[hj][1]
                            s2j = s2[:, j * BL:(j + 1) * BL]
                            if hp_mode == 'out':
                                o32 = tpool.tile(
                                    [128, BL], F32,
                                    name=f"o32_{rep}_{t}_{hj}",
                                    tag="o32", bufs=2)
                                nc.vector.tensor_mul(o32[:], goj, s2j)
                                nc.sync.dma_start(
                                    out=out_d[hj * 128:(hj + 1) * 128, :],
                                    in_=o32[:])
                            elif hp_mode == 'h8':
                                nc.vector.tensor_mul(
                                    h_new[:, 0, hj, :], goj, s2j)
                            elif hp_mode == 'h16':
                                nc.vector.scalar_tensor_tensor(
                                    h_new[:, 0, hj, :], goj, 16.0, s2j,
                                    MULT, MULT)
                            else:  # h3: h16, dh16, h1
                                hf = tpool.tile([128, BL], F32,
                                                name=f"hf_{rep}_{t}_{hj}",
                                                tag="hf", bufs=2)
                                nc.vector.scalar_tensor_tensor(
                                    hf[:], goj, 16.0, s2j, MULT, MULT)
                                nc.vector.tensor_copy(
                                    h_new[:, 0, hj, :], hf[:])
                                nc.vector.tensor_sub(
                                    h_new[:, 1, hj, :], hf[:],
                                    h_new[:, 0, hj, :])
                                nc.vector.tensor_mul(
                                    h_new[:, 2, hj, :], goj, s2j)

                    for ht in range(KH):
                        gates = gpool.tile([128, 4 * BL], BF16,
                                           name=f"gates_{rep}_{t}_{ht}",
                                           tag="g")
                        skip0 = 1 if t == 0 else 0  # f gate unused at t=0
                        pss = [(pspool.tile([128, BL], F32,
                                            name=f"ps_{rep}_{t}_{gi * KH + ht}",
                                            tag="ps")
                                if gi >= skip0 else None)
                               for gi in range(4)]

                        def mm_group(gis):
                            for gi in gis:
                                gt = gi * KH + ht
                                first = True
                                # x terms: (x slot, weight tile)
                                xterms = ([(0, w8_sb)] if xm == '1p'
                                          else [(0, w8_sb), (1, w8_sb),
                                                (2, wl_sb)])
                                for slot, wsb in xterms:
                                    for k2 in range(KD // 2):
                                        nc.tensor.matmul(
                                            pss[gi][:],
                                            wsb[:, k2,
                                                gt * 256:(gt + 1) * 256],
                                            x_t[:, slot,
                                                2 * k2:2 * k2 + 2, :],
                                            start=first,
                                            stop=(t == 0 and
                                                  slot == xterms[-1][0] and
                                                  k2 == KD // 2 - 1),
                                            perf_mode=DR)
                                        first = False
                            if t > 0:
                                hterms = ([(0, u8_sb)] if hm == '1p'
                                          else [(0, u8_sb), (1, u8_sb),
                                                (2, ul_sb)])
                                # kh-major across gates: the freshest h
                                # slices of the previous step are consumed
                                # last.
                                for slot, usb in hterms:
                                    for k2 in range(KH // 2):
                                        for gi in gis:
                                            gt = gi * KH + ht
                                            nc.tensor.matmul(
                                                pss[gi][:],
                                                usb[:, k2,
                                                    gt * 256:
                                                    (gt + 1) * 256],
                                                h_prev[:, slot,
                                                       2 * k2:2 * k2 + 2,
                                                       :],
                                                start=False,
                                                stop=(slot ==
                                                      hterms[-1][0] and
                                                      k2 == KH // 2 - 1),
                                                perf_mode=DR)

                        mm_group([1, 3])
                        mm_group([0, 2] if t > 0 else [2])
                        for gi in (1, 3):
                            gt = gi * KH + ht
                            nc.scalar.activation(
                                gates[:, gi * BL:(gi + 1) * BL],
                                pss[gi][:].bitcast(BF16)[:, 1::2],
                                AF.Tanh if gi == 3 else AF.Sigmoid,
                                bias=b_sb[:, gt:gt + 1], scale=scale)
                        pz = tpool.tile([128, 2 * BL], BF16,
                                        name=f"pz_{rep}_{t}_{ht}", tag="pz",
                                        bufs=3)
                        for j, gi in enumerate((0, 2)):
                            if gi < skip0:
                                continue
                            gt = gi * KH + ht
                            nc.vector.tensor_copy(
                                pz[:, j * BL:(j + 1) * BL], pss[gi][:])
                            nc.scalar.activation(
                                gates[:, gi * BL:(gi + 1) * BL],
                                pz[:, j * BL:(j + 1) * BL],
                                AF.Sigmoid, bias=b_sb[:, gt:gt + 1],
                                scale=scale)
                        gf = gates[:, 0 * BL:1 * BL]
                        gi_ = gates[:, 1 * BL:2 * BL]
                        go = gates[:, 2 * BL:3 * BL]
                        gc = gates[:, 3 * BL:4 * BL]
                        cs = c_sb[:, ht * BL:(ht + 1) * BL]
                        tmp = tpool.tile([128, 2 * BL], BF16,
                                         name=f"tmp_{rep}_{t}_{ht}",
                                         tag="tmp", bufs=4)
                        t1 = tmp[:, 0 * BL:1 * BL]
                        t2 = tmp[:, 1 * BL:2 * BL]
                        slices[ht] = (gf, go, t1, t2)
                        # partial c-update now; the f*c tail one slice
                        # later (act_f flows through the copy path and
                        # would head-block the DVE queue otherwise)
                        if t == 0:
                            nc.vector.tensor_mul(cs, gi_, gc)
                        else:
                            nc.vector.tensor_mul(t2, gi_, gc)
                            if ht >= 1:
                                c_tail(ht - 1)
                        # h-production runs ~3 slices late; the next step
                        # consumes h pairs kh-major in the same order
                        # they finish here.
                        if ht >= 5 - 2 * skip0 and ht % 2 == 1:
                            h_pair((ht - 5 + 2 * skip0) // 2)
                    if t > 0:
                        c_tail(KH - 1)
                        h_pair(2)
                        h_pair(3)
                    else:
                        h_pair(3)
                    if nodep:
                        h_prev = h_fake3 if hprod(t) == 'h3' else h_fake1
                    else:
                        h_prev = h_new

            if reps == 1:
                body(0)
            else:
                with tc.For_i(0, reps, 1):
                    body(0)
    return nc


_BUILT = None


def _get_built():
    global _BUILT
    if _BUILT is None:
        nc = bacc.Bacc("TRN2", num_devices=NCORES)
        build_lstm(nc)
        nc.compile()
        _BUILT = nc
    return _BUILT


def _prep_inputs(x, wf, wi, wo, wc, uf, ui, uo, uc, bf, bi, bo, bc):
    e4 = ml_dtypes.float8_e4m3
    W = np.concatenate([wf, wi, wo, wc], axis=1)                   # [D, 4H]
    U = np.concatenate([uf, ui, uo, uc], axis=1)                   # [H, 4H]
    b = np.concatenate([bf, bi, bo, bc], axis=1).astype(np.float32)
    b_t = np.ascontiguousarray(b.reshape(NGT, 128).T)              # [128, NGT]

    def wimg(M, k):
        """DoubleRowSwInterleave image: per gate-tile block the 256
        columns are (m reversed, A/B interleaved)."""
        Wr = np.asarray(M).reshape(k // 2, 2, 128, NGT, 128)
        Wi = Wr[:, :, :, :, ::-1]                  # [kk, i, p, gt, m]
        img = Wi.transpose(2, 0, 3, 4, 1)          # [p, kk, gt, m, i]
        return np.ascontiguousarray(img.reshape(128, k * G)).astype(e4)
    Wq = W.astype(e4).astype(np.float32)
    Uq = U.astype(e4).astype(np.float32)
    w8 = wimg(W, KD)
    u8 = wimg(U, KH)
    wl = wimg(16.0 * (W - Wq), KD)
    ul = wimg(16.0 * (U - Uq), KH)
    # x [B, T, D] -> per-core [T*D, BL] with xT[t*D+d, b] = x[b, t, d]
    xt = np.ascontiguousarray(np.transpose(x, (1, 2, 0)))  # [T, D, B] f32
    in_maps = []
    for c in range(NCORES):
        xc = np.ascontiguousarray(
            xt[:, :, c * BL:(c + 1) * BL].reshape(T * D, BL))
        x8 = xc.astype(e4)
        x16 = (xc * 16.0).astype(e4)
        xl16 = (xc * 16.0 - x16.astype(np.float32)).astype(e4)
        in_maps.append({"x8": x8, "x16": x16, "xl16": xl16,
                        "w8": w8, "u8": u8, "wl": wl, "ul": ul,
                        "b": b_t})
    return in_maps


def kernel(x, wf, wi, wo, wc, uf, ui, uo, uc, bf, bi, bo, bc):
    nc = _get_built()
    in_maps = _prep_inputs(x, wf, wi, wo, wc, uf, ui, uo, uc, bf, bi, bo, bc)
    res = run_bass_kernel_spmd(nc, in_maps, list(range(NCORES)))
    out = np.empty((B, H), np.float32)
    for c in range(NCORES):
        out[c * BL:(c + 1) * BL, :] = res.results[c]["h_out"].T
    return out
